# revision 24
# baseline (speedup 1.0000x reference)
"""Binary linear layer (sign(x) @ sign(w)) on 8 trn2 NeuronCores.

Strategy
--------
Data-parallel: x is split into 8 row-blocks of 1024; the 4096x4096 weight is
replicated. Each core computes out_shard = sign(x_shard) @ sign(w).

All products are +/-1 and row sums are integers <= 4096, so the matmul is
exact in low precision with fp32 PSUM accumulation. The host ships both
operands already binarized to +/-1 in fp8e4 (a sign(v) re-encode is exactly
as lossy as the sign-exact fp8 cast it replaces, and fp8 transports +/-1
exactly), with each x shard pre-transposed to [d_in, n_per] so the PE
contraction dim lands on SBUF partitions. 20 MB HBM in per core.

The device program is then a pure fp8 DoubleRow GEMM (2 virtual PE rows per
cell): no on-device binarize chains at all, so the first real matmul is
gated only by the first DMA slices. Outputs are integers, evicted
PSUM->SBUF as fp16 (exact to well past the tolerance; |out| <= 4096,
typical |out| ~200) and DMA'd out as 8 MB instead of 16.

Schedule: warmup matmuls on a memset tile burn the DMA-landing latency at
half clock so the HAM un-throttles before the first real matmul; all input
DMAs issue from the sync HW-DGE queue in strict need-order (fine slices at
the front) -- gpsimd initiation uses the slow software DGE path and a
second queue lets far-future slices jump ahead of critical early ones.
n-chunk 0 runs kt-outer across all 8 PSUM banks so the PE paces behind the
streaming x DMA; later chunks run mt-outer with staggered evictions
(scalar queue). The kernel's final m-tile runs as two half-width
accumulation groups so the exit barrier waits only on a short final DMA.

Measured ~238 us (vs ~246 us for the on-device-binarize version). The
matmul stream itself is 1024 DR matmuls x ~216 ns = ~221 us -- the PE
streaming floor for this shape -- plus ~7 us of fixed NEFF semaphore-reset
epilogue and ~5 us of preamble/warmup/tail.
"""

import numpy as np
import ml_dtypes

N_TOTAL, D_IN, D_OUT = 8192, 4096, 4096
N_CORES = 8
N_PER = N_TOTAL // N_CORES

_PROGRAM_CACHE = {}


def build_program(n_per=N_PER, d_in=D_IN, d_out=D_OUT, num_devices=N_CORES):
    """Build + compile the SPMD Bass program (same program on every core)."""
    from concourse import bacc, mybir, tile
    from concourse.bass import ds

    F32 = mybir.dt.float32
    F16 = mybir.dt.float16
    FP8 = mybir.dt.float8e4
    P = 128
    NW = 512            # n-chunk width = one PSUM bank of fp32
    KT = d_in // P      # k-tiles (32)
    MT = n_per // P     # m-tiles per core (8)
    NCH = d_out // NW   # n-chunks (8)
    NK = KT // 2        # DR matmuls per accumulation group (16)
    Copy = mybir.ActivationFunctionType.Copy
    perf_mode = mybir.MatmulPerfMode.DoubleRow

    nc = bacc.Bacc(
        "TRN2",
        target_bir_lowering=False,
        debug=False,
        enable_asserts=False,
        num_devices=num_devices,
    )
    xt = nc.declare_dram_parameter("xt", [d_in, n_per], FP8, isOutput=False)
    w = nc.declare_dram_parameter("w", [d_in, d_out], FP8, isOutput=False)
    out = nc.declare_dram_parameter("out", [n_per, d_out], F16, isOutput=True)

    # HBM-side access patterns with the k-tile index folded into partitions.
    xt_r = xt.ap().rearrange("(kt p) m -> p kt m", p=P)        # [128, KT, n_per]
    w_r = w.ap().rearrange("(kt p) n -> p kt n", p=P)          # [128, KT, d_out]

    with tile.TileContext(nc) as tc:
        with (
            tc.tile_pool(name="xpool", bufs=1) as xpool,
            tc.tile_pool(name="wpool", bufs=4) as wpool,
            tc.tile_pool(name="opool", bufs=8) as opool,
            tc.tile_pool(name="psum", bufs=8, space="PSUM") as pspool,
        ):
            xb = xpool.tile([P, KT * n_per], FP8, tag="xb")
            xb3 = xb[:, :].rearrange("p (kt m) -> p kt m", kt=KT)

            # k-tile slice boundaries for the startup DMAs: fine-grained at
            # the front so the first matmuls are gated on the smallest
            # possible transfer, coarse at the back to keep issue count low.
            # Only sync (SP) and scalar (Activation) have hardware DGEs —
            # gpsimd DMA initiation goes through the slow software path.
            def x_dma(lo, n, eng):
                ktsl = ds(lo, n)
                eng.dma_start(out=xb3[:, ktsl, :], in_=xt_r[:, ktsl, :])

            w_tiles = {}

            def alloc_w(nt):
                w_tiles[nt] = wpool.tile(
                    [P, KT * NW], FP8, tag="wb", name=f"wb{nt}"
                )

            def load_w(nt, lo, n, eng=None):
                nsl = ds(nt * NW, NW)
                wb3 = w_tiles[nt][:, :].rearrange("p (kt n) -> p kt n", kt=KT)
                hsl = ds(lo, n)
                (eng or nc.sync).dma_start(out=wb3[:, hsl, :], in_=w_r[:, hsl, nsl])

            def mm(ps, mt, t, wb3, start, stop):
                nc.tensor.matmul(
                    ps[:, :],
                    lhsT=xb3[:, 2 * t : 2 * t + 2, ds(mt * P, P)],
                    rhs=wb3[:, 2 * t : 2 * t + 2, :],
                    start=start, stop=stop, perf_mode=perf_mode,
                )

            def evict(ps, mt, nt, width=NW, n_off=0, eng=None):
                # ACT copies PSUM fp32 -> SBUF fp16 (values are integers
                # <= 4096: exact to ~5e-4 worst case), then the out DMA is
                # issued from the scalar HW-DGE queue right behind the copy.
                ot = opool.tile([P, width], F16, tag="ot")
                nc.scalar.activation(ot[:, :], ps[:, :width], Copy, 0.0, 1.0)
                (eng or nc.scalar).dma_start(
                    out=out[ds(mt * P, P), ds(nt * NW + n_off, width)],
                    in_=ot[:, :],
                )

            # HAM warmup: dummy matmuls on a memset tile burn the PE-idle
            # time while the first DMA slices land, so the activity monitor
            # un-throttles the PE clock before the real stream begins. They
            # write into ps0[0], which the real k-group overwrites with
            # start=True.
            ps0 = [
                pspool.tile([P, NW], F32, tag="ps", name=f"ps0_{i}")
                for i in range(MT)
            ]
            WARM_MMS = 42
            warm = xpool.tile([P, P], FP8, tag="warm", name="warm")
            nc.gpsimd.memset(warm[:, :], 1.0)
            for _ in range(WARM_MMS):
                nc.tensor.matmul(
                    ps0[0][:, :P], lhsT=warm[:, :], rhs=warm[:, :],
                    start=True, stop=True,
                )

            # Startup DMAs: everything on the sync HW-DGE queue in strict
            # need-order (t-row r consumes k-tile pair 2r,2r+1 every
            # ~1.73us), fine slices at the front so the first matmuls gate
            # on the smallest possible transfer. A second issue queue was
            # measured WORSE (dual-queue interleave let far-future slices
            # race critical early pairs and added a mid-stream stall);
            # supply (~300 GB/s, HBM-bound across all 8 cores) exceeds
            # demand (~220 GB/s), so one strictly ordered queue is
            # stall-free once the first pair lands.
            alloc_w(0)
            for lo, n in ((0, 2), (2, 2), (4, 2), (6, 2),
                          (8, 2), (10, 2), (12, 2), (14, 2),
                          (16, 4), (20, 4), (24, 4), (28, 4)):
                x_dma(lo, n, nc.sync)
                load_w(0, lo, n)

            # n-chunk 0: kt-outer across all MT psum banks, pacing the PE
            # behind the streaming x DMA instead of stalling on full x.
            wb3_0 = w_tiles[0][:, :].rearrange("p (kt n) -> p kt n", kt=KT)
            for t in range(NK):
                for mt in range(MT):
                    mm(ps0[mt], mt, t, wb3_0, start=(t == 0), stop=(t == NK - 1))
            for mt in range(MT):
                evict(ps0[mt], mt, 0)

            # n-chunks 1..: mt-outer (staggered psum eviction)
            for nt in range(1, NCH):
                alloc_w(nt)
                load_w(nt, 0, KT // 2)
                load_w(nt, KT // 2, KT // 2)
                wb3 = w_tiles[nt][:, :].rearrange(
                    "p (kt n) -> p kt n", kt=KT
                )
                last_mt = MT - 1 if nt == NCH - 1 else MT
                for mt in range(last_mt):
                    ps = pspool.tile([P, NW], F32, tag="ps")
                    for t in range(NK):
                        mm(ps, mt, t, wb3, start=(t == 0), stop=(t == NK - 1))
                    evict(ps, mt, nt)

            # The kernel's very last m-tile runs as two half-width (N=256)
            # accumulation groups so the first half's eviction (copy + DMA)
            # fully overlaps the second half's matmuls, and the exit barrier
            # waits only on a short 64KB DMA issued right after the final
            # matmul.
            nt, mt = NCH - 1, MT - 1
            HF = NW // 2
            for h in range(2):
                ps = pspool.tile([P, NW], F32, tag="ps", name=f"psh{h}")
                for t in range(NK):
                    nc.tensor.matmul(
                        ps[:, :HF],
                        lhsT=xb3[:, 2 * t : 2 * t + 2, ds(mt * P, P)],
                        rhs=wb3[:, 2 * t : 2 * t + 2, ds(h * HF, HF)],
                        start=(t == 0), stop=(t == NK - 1),
                        perf_mode=perf_mode,
                    )
                evict(ps, mt, nt, width=HF, n_off=h * HF,
                      eng=nc.sync if h else nc.scalar)

    nc.compile()
    return nc


def _get_program():
    key = (N_PER, D_IN, D_OUT)
    if key not in _PROGRAM_CACHE:
        _PROGRAM_CACHE[key] = build_program()
    return _PROGRAM_CACHE[key]


def shard_inputs(x, weight):
    """Host-side sharding/layout: binarize to +/-1 fp8 + per-shard transpose.

    sign semantics match the reference exactly: v >= 0 -> +1 (including
    +/-0.0), else -1. fp8e4m3 represents +/-1 exactly, so the device GEMM
    is bit-exact integer arithmetic in fp32 PSUM.
    """
    f8 = ml_dtypes.float8_e4m3
    one = np.float32(1.0)
    xe = np.where(np.asarray(x) >= 0, one, -one).astype(f8)
    we = np.where(np.asarray(weight) >= 0, one, -one).astype(f8)
    we = np.ascontiguousarray(we)
    shards = [
        np.ascontiguousarray(xe[i * N_PER : (i + 1) * N_PER].T)
        for i in range(N_CORES)
    ]
    return [{"xt": shards[i], "w": we} for i in range(N_CORES)]


def kernel(x, weight):
    from concourse.bass_utils import run_bass_kernel_spmd

    nc = _get_program()
    in_maps = shard_inputs(np.asarray(x), np.asarray(weight))
    res = run_bass_kernel_spmd(nc, in_maps, list(range(N_CORES)))
    return np.concatenate(
        [res.results[i]["out"] for i in range(N_CORES)], axis=0
    ).astype(np.float32)


# revision 25
# speedup vs baseline: 1.0133x; 1.0133x over previous
"""Binary linear layer (sign(x) @ sign(w)) on 8 trn2 NeuronCores.

Strategy
--------
Data-parallel: x is split into 8 row-blocks of 1024; the 4096x4096 weight is
replicated. Each core computes out_shard = sign(x_shard) @ sign(w).

All products are +/-1 and row sums are integers <= 4096, so the matmul is
exact in low precision with fp32 PSUM accumulation. The host ships both
operands already binarized to +/-1 in fp8e4 (a sign(v) re-encode is exactly
as lossy as the sign-exact fp8 cast it replaces, and fp8 transports +/-1
exactly), with each x shard pre-transposed to [d_in, n_per] so the PE
contraction dim lands on SBUF partitions. 20 MB HBM in per core.

The device program is then a pure fp8 DoubleRow GEMM (2 virtual PE rows per
cell): no on-device binarize chains at all, so the first real matmul is
gated only by the first DMA slices. Outputs are integers, evicted
PSUM->SBUF as fp16 (exact to well past the tolerance; |out| <= 4096,
typical |out| ~200) and DMA'd out as 8 MB instead of 16.

Schedule: warmup matmuls on a memset tile burn the DMA-landing latency at
half clock so the HAM un-throttles before the first real matmul; all input
DMAs issue from the sync HW-DGE queue in strict need-order (fine slices at
the front) -- gpsimd initiation uses the slow software DGE path and a
second queue lets far-future slices jump ahead of critical early ones.
n-chunk 0 runs kt-outer across all 8 PSUM banks so the PE paces behind the
streaming x DMA; later chunks run mt-outer with staggered evictions
(scalar queue). The kernel's final m-tile runs as two half-width
accumulation groups so the exit barrier waits only on a short final DMA.

Measured ~238 us (vs ~246 us for the on-device-binarize version). The
matmul stream itself is 1024 DR matmuls x ~216 ns = ~221 us -- the PE
streaming floor for this shape -- plus ~7 us of fixed NEFF semaphore-reset
epilogue and ~5 us of preamble/warmup/tail.
"""

import numpy as np
import ml_dtypes

N_TOTAL, D_IN, D_OUT = 8192, 4096, 4096
N_CORES = 8
N_PER = N_TOTAL // N_CORES

_PROGRAM_CACHE = {}


def build_program(n_per=N_PER, d_in=D_IN, d_out=D_OUT, num_devices=N_CORES):
    """Build + compile the SPMD Bass program (same program on every core)."""
    from concourse import bacc, mybir, tile
    from concourse.bass import ds

    F32 = mybir.dt.float32
    F16 = mybir.dt.float16
    FP8 = mybir.dt.float8e4
    P = 128
    NW = 512            # n-chunk width = one PSUM bank of fp32
    KT = d_in // P      # k-tiles (32)
    MT = n_per // P     # m-tiles per core (8)
    NCH = d_out // NW   # n-chunks (8)
    NK = KT // 2        # DR matmuls per accumulation group (16)
    Copy = mybir.ActivationFunctionType.Copy
    perf_mode = mybir.MatmulPerfMode.DoubleRow

    nc = bacc.Bacc(
        "TRN2",
        target_bir_lowering=False,
        debug=False,
        enable_asserts=False,
        num_devices=num_devices,
    )
    xt = nc.declare_dram_parameter("xt", [d_in, n_per], FP8, isOutput=False)
    w = nc.declare_dram_parameter("w", [d_in, d_out], FP8, isOutput=False)
    out = nc.declare_dram_parameter("out", [n_per, d_out], F16, isOutput=True)

    # HBM-side access patterns with the k-tile index folded into partitions.
    xt_r = xt.ap().rearrange("(kt p) m -> p kt m", p=P)        # [128, KT, n_per]
    w_r = w.ap().rearrange("(kt p) n -> p kt n", p=P)          # [128, KT, d_out]

    with tile.TileContext(nc) as tc:
        with (
            tc.tile_pool(name="xpool", bufs=1) as xpool,
            tc.tile_pool(name="wpool", bufs=4) as wpool,
            tc.tile_pool(name="opool", bufs=8) as opool,
            tc.tile_pool(name="psum", bufs=8, space="PSUM") as pspool,
        ):
            xb = xpool.tile([P, KT * n_per], FP8, tag="xb")
            xb3 = xb[:, :].rearrange("p (kt m) -> p kt m", kt=KT)

            # k-tile slice boundaries for the startup DMAs: fine-grained at
            # the front so the first matmuls are gated on the smallest
            # possible transfer, coarse at the back to keep issue count low.
            # Only sync (SP) and scalar (Activation) have hardware DGEs —
            # gpsimd DMA initiation goes through the slow software path.
            def x_dma(lo, n, eng):
                ktsl = ds(lo, n)
                eng.dma_start(out=xb3[:, ktsl, :], in_=xt_r[:, ktsl, :])

            w_tiles = {}

            def alloc_w(nt):
                w_tiles[nt] = wpool.tile(
                    [P, KT * NW], FP8, tag="wb", name=f"wb{nt}"
                )

            def load_w(nt, lo, n, eng=None):
                nsl = ds(nt * NW, NW)
                wb3 = w_tiles[nt][:, :].rearrange("p (kt n) -> p kt n", kt=KT)
                hsl = ds(lo, n)
                (eng or nc.sync).dma_start(out=wb3[:, hsl, :], in_=w_r[:, hsl, nsl])

            def mm(ps, mt, t, wb3, start, stop):
                nc.tensor.matmul(
                    ps[:, :],
                    lhsT=xb3[:, 2 * t : 2 * t + 2, ds(mt * P, P)],
                    rhs=wb3[:, 2 * t : 2 * t + 2, :],
                    start=start, stop=stop, perf_mode=perf_mode,
                )

            def evict(ps, mt, nt, width=NW, n_off=0, eng=None):
                # ACT copies PSUM fp32 -> SBUF fp16 (values are integers
                # <= 4096: exact to ~5e-4 worst case), then the out DMA is
                # issued from the scalar HW-DGE queue right behind the copy.
                ot = opool.tile([P, width], F16, tag="ot")
                nc.scalar.activation(ot[:, :], ps[:, :width], Copy, 0.0, 1.0)
                (eng or nc.scalar).dma_start(
                    out=out[ds(mt * P, P), ds(nt * NW + n_off, width)],
                    in_=ot[:, :],
                )

            # HAM warmup: dummy matmuls on a memset tile burn the PE-idle
            # time while the first DMA slices land, so the activity monitor
            # un-throttles the PE clock before the real stream begins. They
            # write into ps0[0], which the real k-group overwrites with
            # start=True.
            ps0 = [
                pspool.tile([P, NW], F32, tag="ps", name=f"ps0_{i}")
                for i in range(MT)
            ]
            WARM_MMS = 46
            warm = xpool.tile([P, P], FP8, tag="warm", name="warm")
            nc.gpsimd.memset(warm[:, :], 1.0)
            for _ in range(WARM_MMS):
                nc.tensor.matmul(
                    ps0[0][:, :P], lhsT=warm[:, :], rhs=warm[:, :],
                    start=True, stop=True,
                )

            # Startup DMAs: everything on the sync HW-DGE queue in strict
            # need-order (t-row r consumes k-tile pair 2r,2r+1 every
            # ~1.73us), fine slices at the front so the first matmuls gate
            # on the smallest possible transfer. A second issue queue was
            # measured WORSE (dual-queue interleave let far-future slices
            # race critical early pairs and added a mid-stream stall);
            # supply (~300 GB/s, HBM-bound across all 8 cores) exceeds
            # demand (~220 GB/s), so one strictly ordered queue is
            # stall-free once the first pair lands.
            alloc_w(0)
            for lo, n in ((0, 2), (2, 2), (4, 2), (6, 2),
                          (8, 2), (10, 2), (12, 2), (14, 2),
                          (16, 4), (20, 4), (24, 4), (28, 4)):
                x_dma(lo, n, nc.sync)
                load_w(0, lo, n)

            # n-chunk 0: kt-outer across all MT psum banks, pacing the PE
            # behind the streaming x DMA instead of stalling on full x.
            wb3_0 = w_tiles[0][:, :].rearrange("p (kt n) -> p kt n", kt=KT)
            for t in range(NK):
                for mt in range(MT):
                    mm(ps0[mt], mt, t, wb3_0, start=(t == 0), stop=(t == NK - 1))
            for mt in range(MT):
                evict(ps0[mt], mt, 0)

            # n-chunks 1..: mt-outer (staggered psum eviction)
            for nt in range(1, NCH):
                alloc_w(nt)
                load_w(nt, 0, KT // 2)
                load_w(nt, KT // 2, KT // 2)
                wb3 = w_tiles[nt][:, :].rearrange(
                    "p (kt n) -> p kt n", kt=KT
                )
                last_mt = MT - 1 if nt == NCH - 1 else MT
                for mt in range(last_mt):
                    ps = pspool.tile([P, NW], F32, tag="ps")
                    for t in range(NK):
                        mm(ps, mt, t, wb3, start=(t == 0), stop=(t == NK - 1))
                    evict(ps, mt, nt)

            # The kernel's very last m-tile runs as two half-width (N=256)
            # accumulation groups so the first half's eviction (copy + DMA)
            # fully overlaps the second half's matmuls, and the exit barrier
            # waits only on a short 64KB DMA issued right after the final
            # matmul.
            nt, mt = NCH - 1, MT - 1
            HF = NW // 2
            for h in range(2):
                ps = pspool.tile([P, NW], F32, tag="ps", name=f"psh{h}")
                for t in range(NK):
                    nc.tensor.matmul(
                        ps[:, :HF],
                        lhsT=xb3[:, 2 * t : 2 * t + 2, ds(mt * P, P)],
                        rhs=wb3[:, 2 * t : 2 * t + 2, ds(h * HF, HF)],
                        start=(t == 0), stop=(t == NK - 1),
                        perf_mode=perf_mode,
                    )
                evict(ps, mt, nt, width=HF, n_off=h * HF,
                      eng=nc.sync if h else nc.scalar)

    nc.compile()
    return nc


def _get_program():
    key = (N_PER, D_IN, D_OUT)
    if key not in _PROGRAM_CACHE:
        _PROGRAM_CACHE[key] = build_program()
    return _PROGRAM_CACHE[key]


def shard_inputs(x, weight):
    """Host-side sharding/layout: binarize to +/-1 fp8 + per-shard transpose.

    sign semantics match the reference exactly: v >= 0 -> +1 (including
    +/-0.0), else -1. fp8e4m3 represents +/-1 exactly, so the device GEMM
    is bit-exact integer arithmetic in fp32 PSUM.
    """
    f8 = ml_dtypes.float8_e4m3
    one = np.float32(1.0)
    xe = np.where(np.asarray(x) >= 0, one, -one).astype(f8)
    we = np.where(np.asarray(weight) >= 0, one, -one).astype(f8)
    we = np.ascontiguousarray(we)
    shards = [
        np.ascontiguousarray(xe[i * N_PER : (i + 1) * N_PER].T)
        for i in range(N_CORES)
    ]
    return [{"xt": shards[i], "w": we} for i in range(N_CORES)]


def kernel(x, weight):
    from concourse.bass_utils import run_bass_kernel_spmd

    nc = _get_program()
    in_maps = shard_inputs(np.asarray(x), np.asarray(weight))
    res = run_bass_kernel_spmd(nc, in_maps, list(range(N_CORES)))
    return np.concatenate(
        [res.results[i]["out"] for i in range(N_CORES)], axis=0
    ).astype(np.float32)


# revision 26
# speedup vs baseline: 1.0149x; 1.0016x over previous
"""Binary linear layer (sign(x) @ sign(w)) on 8 trn2 NeuronCores.

Strategy
--------
Data-parallel: x is split into 8 row-blocks of 1024; the 4096x4096 weight is
replicated. Each core computes out_shard = sign(x_shard) @ sign(w).

All products are +/-1 and row sums are integers <= 4096, so the matmul is
exact in low precision with fp32 PSUM accumulation. The host ships both
operands already binarized to +/-1 in fp8e4 (a sign(v) re-encode is exactly
as lossy as the sign-exact fp8 cast it replaces, and fp8 transports +/-1
exactly), with each x shard pre-transposed to [d_in, n_per] so the PE
contraction dim lands on SBUF partitions. 20 MB HBM in per core.

The device program is then a pure fp8 DoubleRow GEMM (2 virtual PE rows per
cell): no on-device binarize chains at all, so the first real matmul is
gated only by the first DMA slices. Outputs are integers, evicted
PSUM->SBUF as fp16 (exact to well past the tolerance; |out| <= 4096,
typical |out| ~200) and DMA'd out as 8 MB instead of 16.

Schedule: warmup matmuls on a memset tile burn the DMA-landing latency at
half clock so the HAM un-throttles before the first real matmul; all input
DMAs issue from the sync HW-DGE queue in strict need-order (fine slices at
the front) -- gpsimd initiation uses the slow software DGE path and a
second queue lets far-future slices jump ahead of critical early ones.
n-chunk 0 runs kt-outer across all 8 PSUM banks so the PE paces behind the
streaming x DMA; later chunks run mt-outer with staggered evictions
(scalar queue). The kernel's final m-tile runs as two half-width
accumulation groups so the exit barrier waits only on a short final DMA.

Measured ~238 us (vs ~246 us for the on-device-binarize version). The
matmul stream itself is 1024 DR matmuls x ~216 ns = ~221 us -- the PE
streaming floor for this shape -- plus ~7 us of fixed NEFF semaphore-reset
epilogue and ~5 us of preamble/warmup/tail.
"""

import numpy as np
import ml_dtypes

N_TOTAL, D_IN, D_OUT = 8192, 4096, 4096
N_CORES = 8
N_PER = N_TOTAL // N_CORES

_PROGRAM_CACHE = {}


def build_program(n_per=N_PER, d_in=D_IN, d_out=D_OUT, num_devices=N_CORES):
    """Build + compile the SPMD Bass program (same program on every core)."""
    from concourse import bacc, mybir, tile
    from concourse.bass import ds

    F32 = mybir.dt.float32
    F16 = mybir.dt.float16
    FP8 = mybir.dt.float8e4
    P = 128
    NW = 512            # n-chunk width = one PSUM bank of fp32
    KT = d_in // P      # k-tiles (32)
    MT = n_per // P     # m-tiles per core (8)
    NCH = d_out // NW   # n-chunks (8)
    NK = KT // 2        # DR matmuls per accumulation group (16)
    Copy = mybir.ActivationFunctionType.Copy
    perf_mode = mybir.MatmulPerfMode.DoubleRow

    nc = bacc.Bacc(
        "TRN2",
        target_bir_lowering=False,
        debug=False,
        enable_asserts=False,
        num_devices=num_devices,
    )
    xt = nc.declare_dram_parameter("xt", [d_in, n_per], FP8, isOutput=False)
    w = nc.declare_dram_parameter("w", [d_in, d_out], FP8, isOutput=False)
    out = nc.declare_dram_parameter("out", [n_per, d_out], F16, isOutput=True)

    # HBM-side access patterns with the k-tile index folded into partitions.
    xt_r = xt.ap().rearrange("(kt p) m -> p kt m", p=P)        # [128, KT, n_per]
    w_r = w.ap().rearrange("(kt p) n -> p kt n", p=P)          # [128, KT, d_out]

    with tile.TileContext(nc) as tc:
        with (
            tc.tile_pool(name="xpool", bufs=1) as xpool,
            tc.tile_pool(name="wpool", bufs=4) as wpool,
            tc.tile_pool(name="opool", bufs=8) as opool,
            tc.tile_pool(name="psum", bufs=8, space="PSUM") as pspool,
        ):
            xb = xpool.tile([P, KT * n_per], FP8, tag="xb")
            xb3 = xb[:, :].rearrange("p (kt m) -> p kt m", kt=KT)

            # k-tile slice boundaries for the startup DMAs: fine-grained at
            # the front so the first matmuls are gated on the smallest
            # possible transfer, coarse at the back to keep issue count low.
            # Only sync (SP) and scalar (Activation) have hardware DGEs —
            # gpsimd DMA initiation goes through the slow software path.
            def x_dma(lo, n, eng):
                ktsl = ds(lo, n)
                eng.dma_start(out=xb3[:, ktsl, :], in_=xt_r[:, ktsl, :])

            w_tiles = {}

            def alloc_w(nt):
                w_tiles[nt] = wpool.tile(
                    [P, KT * NW], FP8, tag="wb", name=f"wb{nt}"
                )

            def load_w(nt, lo, n, eng=None):
                nsl = ds(nt * NW, NW)
                wb3 = w_tiles[nt][:, :].rearrange("p (kt n) -> p kt n", kt=KT)
                hsl = ds(lo, n)
                (eng or nc.sync).dma_start(out=wb3[:, hsl, :], in_=w_r[:, hsl, nsl])

            def mm(ps, mt, t, wb3, start, stop):
                nc.tensor.matmul(
                    ps[:, :],
                    lhsT=xb3[:, 2 * t : 2 * t + 2, ds(mt * P, P)],
                    rhs=wb3[:, 2 * t : 2 * t + 2, :],
                    start=start, stop=stop, perf_mode=perf_mode,
                )

            def evict(ps, mt, nt, width=NW, n_off=0, eng=None):
                # ACT copies PSUM fp32 -> SBUF fp16 (values are integers
                # <= 4096: exact to ~5e-4 worst case), then the out DMA is
                # issued from the scalar HW-DGE queue right behind the copy.
                ot = opool.tile([P, width], F16, tag="ot")
                nc.scalar.activation(ot[:, :], ps[:, :width], Copy, 0.0, 1.0)
                (eng or nc.scalar).dma_start(
                    out=out[ds(mt * P, P), ds(nt * NW + n_off, width)],
                    in_=ot[:, :],
                )

            # HAM warmup: dummy matmuls on a memset tile burn the PE-idle
            # time while the first DMA slices land, so the activity monitor
            # un-throttles the PE clock before the real stream begins. They
            # write into ps0[0], which the real k-group overwrites with
            # start=True.
            ps0 = [
                pspool.tile([P, NW], F32, tag="ps", name=f"ps0_{i}")
                for i in range(MT)
            ]
            WARM_MMS = 42
            warm = xpool.tile([P, P], FP8, tag="warm", name="warm")
            nc.gpsimd.memset(warm[:, :], 1.0)
            for _ in range(WARM_MMS):
                nc.tensor.matmul(
                    ps0[0][:, :P], lhsT=warm[:, :], rhs=warm[:, :],
                    start=True, stop=True,
                )

            # Startup DMAs: everything on the sync HW-DGE queue in strict
            # need-order (t-row r consumes k-tile pair 2r,2r+1 every
            # ~1.73us), fine slices at the front so the first matmuls gate
            # on the smallest possible transfer. A second issue queue was
            # measured WORSE (dual-queue interleave let far-future slices
            # race critical early pairs and added a mid-stream stall);
            # supply (~300 GB/s, HBM-bound across all 8 cores) exceeds
            # demand (~220 GB/s), so one strictly ordered queue is
            # stall-free once the first pair lands.
            alloc_w(0)
            for lo, n in ((0, 2), (2, 2), (4, 2), (6, 2),
                          (8, 2), (10, 2), (12, 2), (14, 2),
                          (16, 4), (20, 4), (24, 4), (28, 4)):
                x_dma(lo, n, nc.sync)
                load_w(0, lo, n)

            # n-chunk 0: kt-outer across all MT psum banks, pacing the PE
            # behind the streaming x DMA instead of stalling on full x.
            wb3_0 = w_tiles[0][:, :].rearrange("p (kt n) -> p kt n", kt=KT)
            for t in range(NK):
                for mt in range(MT):
                    mm(ps0[mt], mt, t, wb3_0, start=(t == 0), stop=(t == NK - 1))
            for mt in range(MT):
                evict(ps0[mt], mt, 0)

            # n-chunks 1..: mt-outer (staggered psum eviction)
            for nt in range(1, NCH):
                alloc_w(nt)
                load_w(nt, 0, KT // 2)
                load_w(nt, KT // 2, KT // 2)
                wb3 = w_tiles[nt][:, :].rearrange(
                    "p (kt n) -> p kt n", kt=KT
                )
                last_mt = MT - 1 if nt == NCH - 1 else MT
                for mt in range(last_mt):
                    ps = pspool.tile([P, NW], F32, tag="ps")
                    for t in range(NK):
                        mm(ps, mt, t, wb3, start=(t == 0), stop=(t == NK - 1))
                    evict(ps, mt, nt)

            # The kernel's very last m-tile runs as two half-width (N=256)
            # accumulation groups so the first half's eviction (copy + DMA)
            # fully overlaps the second half's matmuls, and the exit barrier
            # waits only on a short 64KB DMA issued right after the final
            # matmul.
            nt, mt = NCH - 1, MT - 1
            HF = NW // 2
            for h in range(2):
                ps = pspool.tile([P, NW], F32, tag="ps", name=f"psh{h}")
                for t in range(NK):
                    nc.tensor.matmul(
                        ps[:, :HF],
                        lhsT=xb3[:, 2 * t : 2 * t + 2, ds(mt * P, P)],
                        rhs=wb3[:, 2 * t : 2 * t + 2, ds(h * HF, HF)],
                        start=(t == 0), stop=(t == NK - 1),
                        perf_mode=perf_mode,
                    )
                evict(ps, mt, nt, width=HF, n_off=h * HF,
                      eng=nc.sync if h else nc.scalar)

    nc.compile()
    return nc


def _get_program():
    key = (N_PER, D_IN, D_OUT)
    if key not in _PROGRAM_CACHE:
        _PROGRAM_CACHE[key] = build_program()
    return _PROGRAM_CACHE[key]


def shard_inputs(x, weight):
    """Host-side sharding/layout: binarize to +/-1 fp8 + per-shard transpose.

    sign semantics match the reference exactly: v >= 0 -> +1 (including
    +/-0.0), else -1. fp8e4m3 represents +/-1 exactly, so the device GEMM
    is bit-exact integer arithmetic in fp32 PSUM.
    """
    f8 = ml_dtypes.float8_e4m3
    one = np.float32(1.0)
    xe = np.where(np.asarray(x) >= 0, one, -one).astype(f8)
    we = np.where(np.asarray(weight) >= 0, one, -one).astype(f8)
    we = np.ascontiguousarray(we)
    shards = [
        np.ascontiguousarray(xe[i * N_PER : (i + 1) * N_PER].T)
        for i in range(N_CORES)
    ]
    return [{"xt": shards[i], "w": we} for i in range(N_CORES)]


def kernel(x, weight):
    from concourse.bass_utils import run_bass_kernel_spmd

    nc = _get_program()
    in_maps = shard_inputs(np.asarray(x), np.asarray(weight))
    res = run_bass_kernel_spmd(nc, in_maps, list(range(N_CORES)))
    return np.concatenate(
        [res.results[i]["out"] for i in range(N_CORES)], axis=0
    ).astype(np.float32)
